# revision 1
# baseline (speedup 1.0000x reference)
"""Trainium2 Bass kernel for nn_Attention_1322849927460.

Dense transformer block: LN -> qkv -> attention (+ spatial-bias MLP on
attention-weighted coordinate deltas) -> out proj -> gelu -> residual.

Sharding: 8 cores = (2 batches) x (4 sequence quarters). Each core holds
all 8 heads for its 512 query rows and the full 2048-token K/V of its
batch, so no collectives are needed. A host-side roll of the token axis
puts each core's query rows first, letting all cores run an identical
SPMD program (attention is invariant to key-order permutation).

Algebraic structure:
  * delta_full[b,h,i,:] = (attn @ xyz)[b,h,i,:] - xyz[b,i,:] since softmax
    rows sum to one -> the (m,m,3) delta tensor is never formed.
  * softmax denominators come free from an augmented V' = [V | xyz | 1]
    contraction; one reciprocal + partition-broadcast normalizes the
    [68, i] accumulator at the end.
  * ln_g and the 1/sqrt(dh) q-scale fold into the qkv weights on host;
    all-zero biases skip their (implemented) device paths.
  * everything stays in "transposed" [feature, token] layout so matmul
    outputs chain straight into the next contraction.
  * matmul operands are bf16 (1 cycle/row + fast weight load); all
    accumulation is fp32 in PSUM, and LayerNorm stats / softmax
    denominators / the final gelu+residual stay fp32.
"""

import os
import sys

for _p in ("/opt/trn_rl_repo",):
    if _p not in sys.path and os.path.isdir(_p):
        sys.path.insert(0, _p)

import ml_dtypes
import numpy as np

import concourse.bass as bass
import concourse.bacc as bacc
import concourse.tile as tile
from concourse import mybir
from concourse.bass_utils import run_bass_kernel_spmd
from concourse.masks import make_identity

F32 = mybir.dt.float32
BF16 = mybir.dt.bfloat16
AF = mybir.ActivationFunctionType
OP = mybir.AluOpType
BF = ml_dtypes.bfloat16

DIM = 256
H = 8
DH = 64
INNER = H * DH  # 512
M = 2048  # tokens per batch
TQ = 512  # query tokens per core
NT = M // 128  # 16 token tiles
N_CORES = 8
LN_EPS = 1e-5


def build_program(has_bqkv: bool, has_spb1: bool, has_spb2: bool):
    nc = bacc.Bacc()

    x_d = nc.dram_tensor("x", [M, DIM], F32, kind="ExternalInput")
    xyza_d = nc.dram_tensor("xyza", [M, 4], BF16, kind="ExternalInput")
    xyzt_d = nc.dram_tensor("xyzt", [3, TQ], BF16, kind="ExternalInput")
    featt_d = nc.dram_tensor("featt", [DIM, TQ], F32, kind="ExternalInput")
    ones_d = nc.dram_tensor("ones", [1, TQ], BF16, kind="ExternalInput")
    wqkv_d = nc.dram_tensor("wqkv", [DIM, 3 * INNER], BF16, kind="ExternalInput")
    bqk_d = nc.dram_tensor("bqk", [128, 8], F32, kind="ExternalInput")
    bv_d = nc.dram_tensor("bv", [1, INNER], BF16, kind="ExternalInput")
    spw1_d = nc.dram_tensor("spw1", [3, 2 * DIM], BF16, kind="ExternalInput")
    spb1_d = nc.dram_tensor("spb1", [128, 4], F32, kind="ExternalInput")
    spw2_d = nc.dram_tensor("spw2", [2 * DIM, DH], BF16, kind="ExternalInput")
    spb2_d = nc.dram_tensor("spb2", [1, DH], BF16, kind="ExternalInput")
    wout_d = nc.dram_tensor("wout", [64, H, DIM], BF16, kind="ExternalInput")
    outb_d = nc.dram_tensor("outb", [128, 2], F32, kind="ExternalInput")
    out_d = nc.dram_tensor("out", [DIM, TQ], F32, kind="ExternalOutput")

    with tile.TileContext(nc) as tc:
        with (
            tc.tile_pool(name="const", bufs=1) as constp,
            tc.tile_pool(name="big", bufs=1) as bigp,
            tc.tile_pool(name="work", bufs=2) as workp,
        ):
            # x first (4 chunks so LN can start early), on the SP queue;
            # weights go via the ACT HWDGE queue.
            x_sb = bigp.tile([128, NT, DIM], F32)
            xv = x_d[:].rearrange("(n p) c -> p n c", p=128)
            for g in range(8):
                eng = nc.sync if g % 2 == 0 else nc.scalar
                eng.dma_start(
                    out=x_sb[:, 2 * g : 2 * g + 2, :],
                    in_=xv[:, 2 * g : 2 * g + 2, :],
                )

            ident = constp.tile([128, 128], BF16)
            make_identity(nc, ident)
            ones_tq = constp.tile([1, TQ], BF16)
            nc.scalar.dma_start(out=ones_tq, in_=ones_d[:])
            eps_t = constp.tile([128, 1], F32)
            nc.vector.memset(eps_t, LN_EPS)

            wqkv_sb = constp.tile([128, 2, 3 * INNER], BF16)
            nc.scalar.dma_start(
                out=wqkv_sb, in_=wqkv_d[:].rearrange("(cc p) o -> p cc o", p=128)
            )
            # spw1 / xyzt live on partitions 64:67 so they base-align with
            # rows 64:67 of the attention accumulator that feed the MLP.
            spw1_sb = constp.tile([67, 2 * DIM], BF16)
            nc.scalar.dma_start(out=spw1_sb[64:67, :], in_=spw1_d[:])
            xyzt_sb = constp.tile([67, TQ], BF16)
            nc.scalar.dma_start(out=xyzt_sb[64:67, :], in_=xyzt_d[:])
            spw2_sb = constp.tile([128, 4, DH], BF16)
            nc.scalar.dma_start(
                out=spw2_sb, in_=spw2_d[:].rearrange("(kc p) d -> p kc d", p=128)
            )
            wout_sb = constp.tile([64, H, DIM], BF16)
            nc.scalar.dma_start(out=wout_sb, in_=wout_d[:])
            outb_sb = constp.tile([128, 2], F32)
            nc.scalar.dma_start(out=outb_sb, in_=outb_d[:])
            featt_sb = constp.tile([128, 2, TQ], F32)
            nc.scalar.dma_start(
                out=featt_sb, in_=featt_d[:].rearrange("(ec p) t -> p ec t", p=128)
            )
            bqk_sb = constp.tile([128, 8], F32)
            nc.scalar.dma_start(out=bqk_sb, in_=bqk_d[:])
            bv_sb = constp.tile([1, INNER], BF16)
            nc.scalar.dma_start(out=bv_sb, in_=bv_d[:])
            spb1_sb = constp.tile([128, 4], F32)
            nc.scalar.dma_start(out=spb1_sb, in_=spb1_d[:])
            spb2_sb = constp.tile([1, DH], BF16)
            nc.scalar.dma_start(out=spb2_sb, in_=spb2_d[:])

            # xyz|ones columns of Vaug, broadcast over heads
            vaug_sb = bigp.tile([128, NT, H, 68], BF16)
            xyza_src = bass.AP(
                tensor=xyza_d,
                offset=0,
                ap=[[4, 128], [512, NT], [1, 4]],
            )
            for h in range(H):
                nc.scalar.dma_start(out=vaug_sb[:, :, h, DH : DH + 4], in_=xyza_src)

            # PE "priming" reads: a dummy matmul per DMA-loaded tile the
            # PE will consume. Each absorbs one DMA-queue semaphore into
            # the PE engine clock (which persists across phases) so real
            # matmuls stay under the per-instruction sync-wait limit.
            ptr_cm = tc.tile_pool(name="ptr", bufs=2, space="PSUM")
            ptr = ptr_cm.__enter__()
            if True:
                # keep the PE spinning through the head so the HAM clock
                # gate opens (K=8/8) and stays open; identity comes from
                # gpsimd so the spam has no DMA dependencies.
                warm_ps = ptr.tile([128, 128], BF16, tag="warm", bufs=1)

                def warm(n):
                    for _ in range(n):
                        nc.tensor.transpose(warm_ps, ident, ident)

                warm(24)
                prime_ps = ptr.tile([4, 4], F32, tag="prime", bufs=1)

                def prime(lhsT, rhs):
                    nc.tensor.matmul(
                        prime_ps[0 : lhsT.shape[-1], 0 : rhs.shape[-1]],
                        lhsT,
                        rhs,
                        start=True,
                        stop=True,
                    )

                prime(wqkv_sb[:, 0, 0:4], wqkv_sb[:, 0, 0:4])
                prime(spw1_sb[64:67, 0:4], spw1_sb[64:67, 0:4])
                prime(spw2_sb[:, 0, 0:4], spw2_sb[:, 0, 0:4])
                prime(wout_sb[:, 0, 0:4], wout_sb[:, 0, 0:4])
                for h in range(H):
                    prime(
                        vaug_sb[:, 0, h, DH : DH + 4],
                        vaug_sb[:, 0, h, DH : DH + 4],
                    )
                if has_bqkv:
                    prime(ones_tq[:, 0:4], bv_sb[:, 0:4])
                if has_spb2:
                    prime(spb2_sb[:, 0:4], ones_tq[:, 0:4])

            # ---- LayerNorm -> xn (bf16) ----
            xn_sb = bigp.tile([128, NT, DIM], BF16)
            mv_all = constp.tile([128, NT, 2], F32)
            for n in range(NT):
                stats = workp.tile([128, 6], F32, tag="bnstats")
                nc.vector.bn_stats(out=stats, in_=x_sb[:, n, :])
                nc.vector.bn_aggr(out=mv_all[:, n, :], in_=stats)
            warm(24)
            rstd = constp.tile([128, NT], F32)
            nc.scalar.activation(
                out=rstd, in_=mv_all[:, :, 1], func=AF.Sqrt, bias=eps_t, scale=1.0
            )
            nc.vector.reciprocal(out=rstd, in_=rstd)
            warm(24)
            for n in range(NT):
                nc.vector.tensor_scalar(
                    out=xn_sb[:, n, :],
                    in0=x_sb[:, n, :],
                    scalar1=mv_all[:, n, 0:1],
                    scalar2=rstd[:, n : n + 1],
                    op0=OP.subtract,
                    op1=OP.mult,
                )

            # ---- transpose xn -> xnT [2 x 128, 2048] ----
            xnt_sb = bigp.tile([128, 2, M], BF16)
            if True:
                for cc in range(2):
                    for nb in range(4):
                        if cc == 0:
                            warm(6)
                        ps = ptr.tile([128, 512], BF16, tag="tr")
                        for q in range(4):
                            n = nb * 4 + q
                            nc.tensor.transpose(
                                ps[:, q * 128 : (q + 1) * 128],
                                xn_sb[:, n, cc * 128 : (cc + 1) * 128],
                                ident,
                            )
                        nc.vector.tensor_copy(
                            xnt_sb[:, cc, nb * 512 : (nb + 1) * 512], ps
                        )
            ptr_cm.__exit__(None, None, None)

            # ---- qT, kT(chunk 0), V ----
            qt_sb = bigp.tile([128, 4, TQ], BF16)
            kt_sb = bigp.tile([128, 4, M], BF16)
            kb_cm = tc.tile_pool(name="kb", bufs=1, space="PSUM")
            kb = kb_cm.__enter__()
            pqv_cm = tc.tile_pool(name="pqv", bufs=2, space="PSUM")
            pqv = pqv_cm.__enter__()
            for g in range(2):  # two groups of two o-chunks
                ps_q = pqv.tile([128, 2, TQ], F32, tag="q", bufs=1)
                for oo in range(2):
                    oc = g * 2 + oo
                    for cc in range(2):
                        nc.tensor.matmul(
                            ps_q[:, oo, :],
                            wqkv_sb[:, cc, oc * 128 : (oc + 1) * 128],
                            xnt_sb[:, cc, 0:TQ],
                            start=(cc == 0),
                            stop=(cc == 1),
                        )
                for oo in range(2):
                    oc = g * 2 + oo
                    if has_bqkv:
                        nc.vector.tensor_scalar(
                            out=qt_sb[:, oc, :],
                            in0=ps_q[:, oo, :],
                            scalar1=bqk_sb[:, oc : oc + 1],
                            scalar2=None,
                            op0=OP.add,
                        )
                    else:
                        nc.vector.tensor_copy(qt_sb[:, oc, :], ps_q[:, oo, :])

            def emit_kt(oc):
                for half in range(2):
                    ps_k = kb.tile([128, 2, TQ], F32, tag="k", bufs=1)
                    for tt in range(2):
                        tb = half * 2 + tt
                        for cc in range(2):
                            nc.tensor.matmul(
                                ps_k[:, tt, :],
                                wqkv_sb[
                                    :, cc, INNER + oc * 128 : INNER + (oc + 1) * 128
                                ],
                                xnt_sb[:, cc, tb * 512 : (tb + 1) * 512],
                                start=(cc == 0),
                                stop=(cc == 1),
                            )
                    if has_bqkv:
                        nc.vector.tensor_scalar(
                            out=kt_sb[:, oc, half * 1024 : (half + 1) * 1024],
                            in0=ps_k,
                            scalar1=bqk_sb[:, 4 + oc : 5 + oc],
                            scalar2=None,
                            op0=OP.add,
                        )
                    else:
                        nc.vector.tensor_copy(
                            kt_sb[:, oc, half * 1024 : (half + 1) * 1024], ps_k
                        )

            emit_kt(0)

            for n in range(NT):
                ps_v = pqv.tile([128, INNER], F32, tag="v", bufs=2)
                for cc in range(2):
                    nc.tensor.matmul(
                        ps_v,
                        xnt_sb[:, cc, n * 128 : (n + 1) * 128],
                        wqkv_sb[:, cc, 2 * INNER : 3 * INNER],
                        start=(cc == 0),
                        stop=(cc == 1 and not has_bqkv),
                    )
                if has_bqkv:
                    nc.tensor.matmul(
                        ps_v, ones_tq[:, 0:128], bv_sb, start=False, stop=True
                    )
                nc.vector.tensor_copy(
                    vaug_sb[:, n, :, 0:DH],
                    ps_v[:].rearrange("p (h d) -> p h d", h=H),
                )
            pqv_cm.__exit__(None, None, None)

            # ---- attention: 4 passes x 2 heads ----
            # Raw (unnormalized) accumulators are evicted per pass; all
            # normalization happens afterwards so the QK->exp->AV stream
            # never waits on DMA round-trips. kT chunks 1-3 are emitted
            # inside earlier passes to fill PE gaps.
            araw_sb = bigp.tile([68, 4, 2, TQ], F32)
            anorm_sb = bigp.tile([68, 4, 2, TQ], BF16)
            rsp_cm = tc.tile_pool(name="rsp", bufs=2)
            rsp = rsp_cm.__enter__()
            with (
                tc.tile_pool(name="pattn", bufs=2, space="PSUM") as pattn,
                tc.tile_pool(name="expp", bufs=3) as expp,
            ):
                for p in range(4):
                    accum = pattn.tile([68, 2, TQ], F32, tag="accum", bufs=1)
                    for j in range(NT):
                        sT = pattn.tile([128, 2, TQ], F32, tag="sT", bufs=2)
                        for hh in range(2):
                            nc.tensor.matmul(
                                sT[:, hh, :],
                                kt_sb[
                                    hh * 64 : hh * 64 + 64,
                                    p,
                                    j * 128 : (j + 1) * 128,
                                ],
                                qt_sb[hh * 64 : hh * 64 + 64, p, :],
                                start=True,
                                stop=True,
                            )
                        e = expp.tile([128, 2, TQ], BF16, tag="exp")
                        nc.scalar.activation(out=e, in_=sT, func=AF.Exp)
                        for hh in range(2):
                            h = 2 * p + hh
                            nc.tensor.matmul(
                                accum[:, hh, :],
                                vaug_sb[:, j, h, :],
                                e[:, hh, :],
                                start=(j == 0),
                                stop=(j == NT - 1),
                            )
                        if j == 5 and p < 3:
                            emit_kt(p + 1)
                    nc.vector.tensor_copy(araw_sb[:, p, :, :], accum)
                    # normalization for this pass runs under the next pass:
                    # everything reads/writes SBUF, no PSUM involvement.
                    rs = rsp.tile([128, 8], F32, tag="rs")
                    nc.sync.dma_start(out=rs, in_=araw_sb[67:68, p, :, :])
                    rc = rsp.tile([128, 8], F32, tag="rc")
                    nc.vector.reciprocal(out=rc, in_=rs)
                    rrow = rsp.tile([1, 2, TQ], F32, tag="rrow")
                    nc.sync.dma_start(out=rrow, in_=rc)
                    for hh in range(2):
                        rbc = rsp.tile([68, TQ], F32, tag="rbc", bufs=3)
                        nc.gpsimd.partition_broadcast(
                            rbc, rrow[0:1, hh, :], channels=68
                        )
                        nc.vector.tensor_tensor(
                            out=anorm_sb[:, p, hh, :],
                            in0=araw_sb[:, p, hh, :],
                            in1=rbc,
                            op=OP.mult,
                        )
                        # rows 64:67 -= xyz_i  (in place, base-64 aligned)
                        nc.vector.tensor_tensor(
                            out=anorm_sb[64:67, p, hh, :],
                            in0=anorm_sb[64:67, p, hh, :],
                            in1=xyzt_sb[64:67, :],
                            op=OP.subtract,
                        )
            kb_cm.__exit__(None, None, None)

            # ---- spatial-bias MLP per head + combine ----
            outfin_sb = bigp.tile([64, H, TQ], BF16)
            with (
                tc.tile_pool(name="pmlp", bufs=1, space="PSUM") as pmlp,
                tc.tile_pool(name="hpool", bufs=2) as hpool,
            ):
                for h in range(H):
                    p, hh = h // 2, h % 2
                    hT = pmlp.tile([128, 4, TQ], F32, tag="h1", bufs=1)
                    for kc in range(4):
                        nc.tensor.matmul(
                            hT[:, kc, :],
                            spw1_sb[64:67, kc * 128 : (kc + 1) * 128],
                            anorm_sb[64:67, p, hh, :],
                            start=True,
                            stop=True,
                        )
                    hsb = hpool.tile([128, 4, TQ], BF16, tag="hsb")
                    if has_spb1:
                        for kc in range(4):
                            nc.scalar.activation(
                                out=hsb[:, kc, :],
                                in_=hT[:, kc, :],
                                func=AF.Gelu,
                                bias=spb1_sb[:, kc : kc + 1],
                            )
                    else:
                        nc.scalar.activation(out=hsb, in_=hT, func=AF.Gelu)
                    sbias = pmlp.tile([64, TQ], F32, tag="sbias", bufs=2)
                    for kc in range(4):
                        nc.tensor.matmul(
                            sbias,
                            spw2_sb[:, kc, :],
                            hsb[:, kc, :],
                            start=(kc == 0),
                            stop=(kc == 3 and not has_spb2),
                        )
                    if has_spb2:
                        nc.tensor.matmul(
                            sbias, spb2_sb, ones_tq, start=False, stop=True
                        )
                    nc.vector.tensor_tensor(
                        out=outfin_sb[:, h, :],
                        in0=anorm_sb[0:64, p, hh, :],
                        in1=sbias,
                        op=OP.add,
                    )

            rsp_cm.__exit__(None, None, None)

            # ---- output projection + gelu + residual (transposed layout) ----
            with tc.tile_pool(name="pproj", bufs=1, space="PSUM") as pproj:
                yT = pproj.tile([128, 2, TQ], F32, tag="y")
                for ec in range(2):
                    for h in range(H):
                        nc.tensor.matmul(
                            yT[:, ec, :],
                            wout_sb[:, h, ec * 128 : (ec + 1) * 128],
                            outfin_sb[:, h, :],
                            start=(h == 0),
                            stop=(h == H - 1),
                        )
                ysb = workp.tile([128, 2, TQ], F32, tag="ysb")
                for ec in range(2):
                    nc.scalar.activation(
                        out=ysb[:, ec, :],
                        in_=yT[:, ec, :],
                        func=AF.Gelu,
                        bias=outb_sb[:, ec : ec + 1],
                    )
                res = workp.tile([128, 2, TQ], F32, tag="res")
                nc.vector.tensor_tensor(out=res, in0=ysb, in1=featt_sb, op=OP.add)
                nc.sync.dma_start(
                    out=out_d[:].rearrange("(ec p) t -> p ec t", p=128), in_=res
                )

    nc.compile()
    return nc


def prepare_maps(inputs):
    xyzs = np.asarray(inputs["xyzs"], np.float32)
    features = np.asarray(inputs["features"], np.float32)
    ln_g = np.asarray(inputs["ln_g"], np.float32)
    ln_b = np.asarray(inputs["ln_b"], np.float32)
    w_qkv = np.asarray(inputs["w_qkv"], np.float32)
    sp_w1 = np.asarray(inputs["sp_w1"], np.float32)
    sp_b1 = np.asarray(inputs["sp_b1"], np.float32)
    sp_w2 = np.asarray(inputs["sp_w2"], np.float32)
    sp_b2 = np.asarray(inputs["sp_b2"], np.float32)
    out_w = np.asarray(inputs["out_w"], np.float32)
    out_b = np.asarray(inputs["out_b"], np.float32)

    scale = DH ** -0.5
    wqkv_f = w_qkv * ln_g[:, None]
    wqkv_f[:, :INNER] = wqkv_f[:, :INNER] * scale
    bqkv = (ln_b @ w_qkv).astype(np.float32)
    bqkv[:INNER] *= scale

    has_bqkv = bool(np.any(bqkv != 0.0))
    has_spb1 = bool(np.any(sp_b1 != 0.0))
    has_spb2 = bool(np.any(sp_b2 != 0.0))

    bqk = np.zeros((128, 8), np.float32)
    for oc in range(4):
        bqk[:, oc] = bqkv[oc * 128 : (oc + 1) * 128]
        bqk[:, 4 + oc] = bqkv[INNER + oc * 128 : INNER + (oc + 1) * 128]
    spb1 = np.zeros((128, 4), np.float32)
    for kc in range(4):
        spb1[:, kc] = sp_b1[kc * 128 : (kc + 1) * 128]
    outb = np.stack([out_b[:128], out_b[128:]], axis=1).astype(np.float32)
    # wout as [64, H, 256]: row (d, h) = out_w[h*64+d, :]
    wout64 = np.ascontiguousarray(out_w.reshape(H, 64, DIM).transpose(1, 0, 2))

    shared = {
        "wqkv": np.ascontiguousarray(wqkv_f).astype(BF),
        "bqk": bqk,
        "bv": np.ascontiguousarray(bqkv[2 * INNER :].reshape(1, INNER)).astype(BF),
        "spw1": np.ascontiguousarray(sp_w1).astype(BF),
        "spb1": spb1,
        "spw2": np.ascontiguousarray(sp_w2).astype(BF),
        "spb2": np.ascontiguousarray(sp_b2.reshape(1, DH)).astype(BF),
        "wout": wout64.astype(BF),
        "outb": outb,
        "ones": np.ones((1, TQ), np.float32).astype(BF),
    }

    in_maps = []
    for core in range(N_CORES):
        bi, quarter = core // 4, core % 4
        qs = quarter * TQ
        x_b = features[bi].reshape(M, DIM)
        xyz_b = xyzs[bi].reshape(M, 3)
        x_perm = np.roll(x_b, -qs, axis=0)
        xyz_perm = np.roll(xyz_b, -qs, axis=0)
        xyza = np.concatenate(
            [xyz_perm, np.ones((M, 1), np.float32)], axis=1
        ).astype(np.float32)
        m = dict(shared)
        m["x"] = np.ascontiguousarray(x_perm)
        m["xyza"] = np.ascontiguousarray(xyza).astype(BF)
        m["xyzt"] = np.ascontiguousarray(xyz_perm[:TQ].T).astype(BF)
        m["featt"] = np.ascontiguousarray(x_perm[:TQ].T)
        in_maps.append(m)
    return in_maps, (has_bqkv, has_spb1, has_spb2)


def assemble(results, l=16, n=128):
    out = np.zeros((2, M, DIM), np.float32)
    for core in range(N_CORES):
        bi, quarter = core // 4, core % 4
        qs = quarter * TQ
        out[bi, qs : qs + TQ, :] = results[core]["out"].T
    return out.reshape(2, l, n, DIM)


def kernel(**inputs):
    in_maps, flags = prepare_maps(inputs)
    nc = build_program(*flags)
    results = run_bass_kernel_spmd(nc, in_maps, list(range(N_CORES))).results
    return assemble(results)


if __name__ == "__main__":
    pass

